# revision 11
# baseline (speedup 1.0000x reference)
"""Chunk-causal Whisper attention (B=4, T=1500, D=1024, H=16) on 8 NeuronCores.

Sharding: core c = (batch b = c//2, head-half hh = c%2). Each core runs one
batch element with 8 of the 16 heads (512 of 1024 channels). All on-chip
tensors are kept transposed: scoresT[k,q] = K @ Q^T per head, so the softmax
reduction runs along the partition (key) axis. A ones-column appended to V
makes the PV matmul produce both the unnormalized output and the softmax
denominator; normalization broadcasts 1/denom across partitions with a gpsimd
partition_broadcast and one DVE multiply. The Wo matmul consumes the
transposed attention output directly; the host sums the two head-half
partials and adds the constant (bv @ Wo + bo) (bv folds through softmax
because the probabilities sum to 1).

All matmul operands are bf16 (PSUM accumulation stays fp32). Score tiles are
exp'd in PAIRS (one scalar-engine activation over two adjacent PSUM banks);
masking is a multiplicative {0,1} bf16 mask applied to exp(scores) on the
vector engine.

Sparsity: the chunk-causal mask is monotone per query, so each (128-key x
query-window) block is classified full / partial / skip, and additionally
each block's matmul+exp+mask+PV moving range is RESTRICTED to the query
window [qlo, qhi) that actually attends any key of the block (-22% attention
cycles vs full 512-wide blocks). The PV accumulation starts with kc=0 (always
full-window) so every PSUM column is initialized before narrower blocks
accumulate into suffix ranges. Query-side padding is clamped to 1504 (vs the
1536 key padding), so qc2 attention / Q tb2 / Wo qc2 / output DMA all run 480
wide.

DMA: each dma_start costs ~0.6us of serialized Sync-engine issue time
regardless of size, so inputs are shipped as a handful of merged rearranged
transfers (weights as one DMA per tensor, hs in 3 pieces) ordered so phase-A
prerequisites land first. Outputs accumulate in per-qc fin tiles (fp16) and
leave in 1-2 merged DMAs per qc; the host upcasts and sums the halves.

Projections run dc-outer/tb-inner over (tb0,tb1) pairs so the PE keeps one
stationary weight block across two 512-wide moving matmuls (halves weight
switches). Attention pairs are software-pipelined with lag 1 as before.
"""

import sys

import numpy as np
import ml_dtypes

if "/opt/trn_rl_repo" not in sys.path:
    sys.path.insert(0, "/opt/trn_rl_repo")

import concourse.tile as tile  # noqa: E402
from concourse import bacc, mybir  # noqa: E402
import concourse.bass_utils as bass_utils  # noqa: E402

B, T, D = 4, 1500, 1024
H, HD = 16, 64
CHUNK, LOOK = 100, 50
TP = 1536          # padded sequence length (key side)
TQ = 1504          # padded query length (32-aligned)
CH = 512           # channels per core (8 heads)
HL = 8             # heads per core
NCORES = 8
SCALE = HD ** -0.5
QB = 512           # query block (matmul moving free dim)
KB = 128           # key block (contraction tile)
NQC = TP // QB     # 3
NKC = TP // KB     # 12
NDC = D // 128     # 8
NCC = CH // 128    # 4
NOC = D // 128     # 8 output-column chunks
Q_HI = [512, 512, TQ - 2 * QB]  # per-qc real query width (480 for qc2)
F32 = mybir.dt.float32
F16 = mybir.dt.float16
BF16 = mybir.dt.bfloat16


def _jmax(i):
    """Largest key index query i may attend to."""
    return max((i // CHUNK) * CHUNK + CHUNK - 1, i + LOOK)


def _classify():
    """Per (qc, kc): ('skip'|'full'|mask-index, qlo). qlo is the first
    (32-aligned) query column in the block that attends any key of kc."""
    status = {}
    masks = []
    for qc in range(NQC):
        q0 = qc * QB
        qhi = Q_HI[qc]
        hi = max(_jmax(min(q0 + i, T - 1)) for i in range(qhi))
        lo = _jmax(q0)
        for kc in range(NKC):
            k0 = kc * KB
            if k0 > hi:
                status[(qc, kc)] = ("skip", 0)
                continue
            qlo = 0
            for ii in range(qhi):
                if _jmax(min(q0 + ii, T - 1)) >= k0:
                    qlo = (ii // 32) * 32
                    break
            if k0 + KB - 1 <= lo and k0 + KB <= T:
                status[(qc, kc)] = ("full", qlo)
                continue
            m = np.zeros((KB, QB), np.float32)
            for ii in range(qlo, qhi):
                i = min(q0 + ii, T - 1)
                n_ok = min(min(_jmax(i), T - 1) + 1 - k0, KB)
                if n_ok > 0:
                    m[:n_ok, ii] = 1.0
            status[(qc, kc)] = (len(masks), qlo)
            masks.append(m)
    return status, masks


_STATUS, _MASKS = _classify()
NPART = len(_MASKS)
NMASK0 = sum(1 for kc in range(NKC)
             if _STATUS[(0, kc)][0] not in ("skip", "full"))  # qc0 masks

# Vx ones-column values: 1.0 for real keys, 0.0 for the padded tail.
_VONES = np.zeros((128, NKC, HL), np.float32)
for _tk in range(NKC):
    for _p in range(128):
        if _tk * KB + _p < T:
            _VONES[_p, _tk, :] = 1.0


def _build():
    nc = bacc.Bacc("TRN2", target_bir_lowering=False, debug=False)
    hsT = nc.dram_tensor("hsT", [D, TP], BF16, kind="ExternalInput")[:]
    wq = nc.dram_tensor("wq", [D, CH], BF16, kind="ExternalInput")[:]
    wk = nc.dram_tensor("wk", [D, CH], BF16, kind="ExternalInput")[:]
    wv = nc.dram_tensor("wv", [D, CH], BF16, kind="ExternalInput")[:]
    bqs = nc.dram_tensor("bqs", [CH], F32, kind="ExternalInput")[:]
    wo = nc.dram_tensor("wo", [CH, D], BF16, kind="ExternalInput")[:]
    maskT = nc.dram_tensor("maskT", [NPART, KB, QB], BF16, kind="ExternalInput")[:]
    vones = nc.dram_tensor("vones", [128, NKC, HL], BF16, kind="ExternalInput")[:]
    outT = nc.dram_tensor("outT", [D, TQ], F16, kind="ExternalOutput")[:]

    hsT_m = hsT.rearrange("(a p) t -> p a t", p=128)
    wq_m = wq.rearrange("(a p) c -> p a c", p=128)
    wk_m = wk.rearrange("(a p) c -> p a c", p=128)
    wv_m = wv.rearrange("(a p) c -> p a c", p=128)
    wo_m = wo.rearrange("(a p) o -> p a o", p=128)
    mk_m = maskT.rearrange("n p q -> p n q")
    outT_m = outT.rearrange("(a p) t -> p a t", p=128)

    ExpF = mybir.ActivationFunctionType.Exp

    with tile.TileContext(nc) as tc, \
         tc.tile_pool(name="per", bufs=1) as per, \
         tc.tile_pool(name="ex", bufs=6) as ep, \
         tc.tile_pool(name="sm", bufs=6) as sp, \
         tc.tile_pool(name="ps_p", bufs=2, space="PSUM") as ps_p, \
         tc.tile_pool(name="ps_s", bufs=2, space="PSUM") as ps_s, \
         tc.tile_pool(name="ps_pv", bufs=2, space="PSUM") as ps_pv:
        KT = per.tile([128, NCC, TP], BF16)        # K^T: [c, cc, t]
        QT = per.tile([128, NCC, TP], BF16)        # Q^T (scale+bias folded)
        Vx = per.tile([128, NKC, HL, HD + 1], BF16)  # V + ones column
        AT = per.tile([128, NCC, TP], BF16)        # normalized attnT
        hs_sb = per.tile([128, NDC, TP], BF16)
        wk_sb = per.tile([128, NDC, CH], BF16)
        wq_sb = per.tile([128, NDC, CH], BF16)
        wv_sb = per.tile([128, NDC, CH], BF16)
        wo_sb = per.tile([128, NCC, D], BF16)
        mk_sb = per.tile([128, NPART, QB], BF16)
        bq_sb = per.tile([128, NCC], F32)
        fin = [per.tile([128, NOC, QB], F16, name=f"fin{qc}")
               for qc in range(NQC)]

        # DMA order: phase-A prerequisites first; each dma_start costs
        # ~0.6us of serialized issue, so transfers are merged per tensor.
        # vones must land before any V-projection copy touches Vx (strided
        # 2-byte writes race with engine writes to adjacent bytes).
        nc.sync.dma_start(bq_sb[:], bqs.rearrange("(a p) -> p a", p=128))
        nc.sync.dma_start(Vx[:, :, :, HD:HD + 1], vones[:, :, :, None])
        # Merged transfers in phase-A-priority order. Starting the PE
        # earlier on a per-plane trickle measured WORSE (repeated p-state
        # ramps); batched arrival keeps the PE saturated once it starts.
        nc.sync.dma_start(hs_sb[:, 0:2, 0:1024], hsT_m[:, 0:2, 0:1024])
        nc.sync.dma_start(hs_sb[:, 2:NDC, 0:1024], hsT_m[:, 2:NDC, 0:1024])
        nc.sync.dma_start(wk_sb[:], wk_m)
        nc.sync.dma_start(wq_sb[:], wq_m)
        nc.sync.dma_start(wv_sb[:], wv_m)
        nc.sync.dma_start(mk_sb[:, 0:NMASK0, :], mk_m[:, 0:NMASK0, :])
        nc.sync.dma_start(hs_sb[:, :, 1024:TP], hsT_m[:, :, 1024:TP])
        nc.sync.dma_start(mk_sb[:, NMASK0:, :], mk_m[:, NMASK0:, :])
        nc.sync.dma_start(wo_sb[:], wo_m)

        # ---- projection unit emitters ----
        def emit_kq(kind, cc, act=False):
            """K or Q projection for column chunk cc over tb0 then tb1."""
            w_sb = wk_sb if kind == "k" else wq_sb
            for tb in (0, 1):
                ps = ps_p.tile([128, QB], F32, tag="p", name=f"{kind}{tb}{cc}")
                for dc in range(NDC):
                    nc.tensor.matmul(
                        ps[:], w_sb[:, dc, cc * 128:(cc + 1) * 128],
                        hs_sb[:, dc, tb * QB:(tb + 1) * QB],
                        start=(dc == 0), stop=(dc == NDC - 1))
                ts = slice(tb * QB, (tb + 1) * QB)
                if kind == "k":
                    if act:
                        nc.scalar.copy(KT[:, cc, ts], ps[:])
                    else:
                        nc.vector.tensor_copy(KT[:, cc, ts], ps[:])
                else:
                    if act:
                        nc.scalar.add(QT[:, cc, ts], ps[:], bq_sb[:, cc:cc + 1])
                    else:
                        nc.vector.tensor_scalar_add(
                            QT[:, cc, ts], ps[:], bq_sb[:, cc:cc + 1])

        def emit_k2(cc):
            ps = ps_p.tile([128, QB], F32, tag="p", name=f"k2{cc}")
            for dc in range(NDC):
                nc.tensor.matmul(
                    ps[:], wk_sb[:, dc, cc * 128:(cc + 1) * 128],
                    hs_sb[:, dc, 1024:TP], start=(dc == 0), stop=(dc == NDC - 1))
            nc.vector.tensor_copy(KT[:, cc, 1024:TP], ps[:])

        def emit_q_range(cc, t0, t1):
            ps = ps_p.tile([128, QB], F32, tag="p", name=f"q2{cc}")
            for dc in range(NDC):
                nc.tensor.matmul(
                    ps[:, 0:t1 - t0], wq_sb[:, dc, cc * 128:(cc + 1) * 128],
                    hs_sb[:, dc, t0:t1], start=(dc == 0), stop=(dc == NDC - 1))
            nc.vector.tensor_scalar_add(
                QT[:, cc, t0:t1], ps[:, 0:t1 - t0], bq_sb[:, cc:cc + 1])

        def emit_v(tk):
            ps = ps_p.tile([128, CH], F32, tag="p", name=f"v{tk}")
            for dc in range(NDC):
                nc.tensor.matmul(
                    ps[:], hs_sb[:, dc, tk * KB:(tk + 1) * KB],
                    wv_sb[:, dc, :], start=(dc == 0), stop=(dc == NDC - 1))
            nc.vector.tensor_copy(
                Vx[:, tk, :, 0:HD], ps[:].rearrange("p (h d) -> p h d", d=HD))

        # ---- attention group: software-pipelined pairs (lag 1) ----
        def attn(h, qc):
            pb = 64 * (h % 2)
            cc = h // 2
            q0 = qc * QB
            w = Q_HI[qc]
            kcs = [kc for kc in range(NKC) if _STATUS[(qc, kc)][0] != "skip"]
            pairs = [kcs[i:i + 2] for i in range(0, len(kcs), 2)]
            nk = len(kcs)
            pv = ps_pv.tile([HD + 1, QB], F32)
            state = [0]

            def emit_pv(pair_kcs, ex, qlos):
                for j, (kc, ql) in enumerate(zip(pair_kcs, qlos)):
                    nc.tensor.matmul(
                        pv[:, ql:w], Vx[:, kc, h, :], ex[:, j, ql:w],
                        start=(state[0] == 0), stop=(state[0] == nk - 1))
                    state[0] += 1

            pending = []
            for i in range(0, len(pairs), 2):
                chunk = pairs[i:i + 2]
                staged = []
                for pair in chunk:
                    qlos = [_STATUS[(qc, kc)][1] for kc in pair]
                    ss = ps_s.tile([128, 2, QB], F32)
                    for j, (kc, ql) in enumerate(zip(pair, qlos)):
                        nc.tensor.matmul(
                            ss[:, j, ql:w],
                            KT[pb:pb + 64, cc, kc * KB:(kc + 1) * KB],
                            QT[pb:pb + 64, cc, q0 + ql:q0 + w],
                            start=True, stop=True)
                    ex = ep.tile([128, 2, QB], BF16, tag="e", name="e")
                    lo = min(qlos)
                    # exp over the union window; narrower plane's head region
                    # holds stale-but-finite data that is never read (PV and
                    # mask both use the per-plane window).
                    if len(pair) == 2:
                        nc.scalar.activation(
                            ex[:, :, lo:w], ss[:, :, lo:w], ExpF)
                    else:
                        nc.scalar.activation(
                            ex[:, 0, lo:w], ss[:, 0, lo:w], ExpF)
                    for j, (kc, ql) in enumerate(zip(pair, qlos)):
                        st = _STATUS[(qc, kc)][0]
                        if st != "full":
                            nc.vector.tensor_mul(
                                ex[:, j, ql:w], ex[:, j, ql:w],
                                mk_sb[:, st, ql:w])
                    staged.append((pair, ex, qlos))
                for pk, e, ql in pending:
                    emit_pv(pk, e, ql)
                pending = staged
            for pk, e, ql in pending:
                emit_pv(pk, e, ql)
            # normalize: 1/denominator broadcast across the head partitions
            dn = sp.tile([1, QB], F32, tag="dn", name="dn")
            nc.vector.tensor_copy(dn[:, 0:w], pv[HD:HD + 1, 0:w])
            rc = sp.tile([1, QB], F32, tag="recip", name="recip")
            nc.vector.reciprocal_approx_fast(rc[:, 0:w], dn[:, 0:w])
            bc = sp.tile([HD, QB], F32, tag="bcast", name="bcast")
            nc.gpsimd.partition_broadcast(bc[:, 0:w], rc[:, 0:w])
            nc.vector.tensor_mul(
                AT[pb:pb + 64, cc, q0:q0 + w], pv[0:HD, 0:w], bc[:, 0:w])

        def phase3(qc, ocs):
            q0 = qc * QB
            w = Q_HI[qc]
            for oc in ocs:
                po = ps_p.tile([128, QB], F32, tag="p", name=f"o{oc}")
                for ccc in range(NCC):
                    nc.tensor.matmul(
                        po[:, 0:w], wo_sb[:, ccc, oc * 128:(oc + 1) * 128],
                        AT[:, ccc, q0:q0 + w],
                        start=(ccc == 0), stop=(ccc == NCC - 1))
                nc.vector.tensor_copy(fin[qc][:, oc, 0:w], po[:, 0:w])

        def phase3_tail(qc):
            """Final Wo pass: cc1-3 accumulate while the last heads (0,1 ->
            cc0) finish their normalize chain; only the closing cc0 matmul
            of each unit waits on it."""
            q0 = qc * QB
            w = Q_HI[qc]

            def stage(oc):
                po = ps_p.tile([128, QB], F32, tag="p", name=f"t{oc}")
                for ccc in (1, 2, 3):
                    nc.tensor.matmul(
                        po[:, 0:w], wo_sb[:, ccc, oc * 128:(oc + 1) * 128],
                        AT[:, ccc, q0:q0 + w], start=(ccc == 1), stop=False)
                return po

            def finish(oc, po):
                nc.tensor.matmul(
                    po[:, 0:w], wo_sb[:, 0, oc * 128:(oc + 1) * 128],
                    AT[:, 0, q0:q0 + w], start=False, stop=True)
                nc.vector.tensor_copy(fin[qc][:, oc, 0:w], po[:, 0:w])

            po = [stage(0), stage(1)]
            for oc in range(NOC):
                finish(oc, po[oc % 2])
                if oc + 2 < NOC:
                    po[oc % 2] = stage(oc + 2)
                if oc % 2 == 1:
                    dma_out(qc, oc - 1, oc + 1)

        def dma_out(qc, oc0, oc1):
            q0 = qc * QB
            w = Q_HI[qc]
            nc.sync.dma_start(
                outT_m[:, oc0:oc1, q0:q0 + w], fin[qc][:, oc0:oc1, 0:w])

        # ---- phase A: minimal head so attention can start early ----
        # K/Q epilogues on ACT, V copies on DVE, so ACT is free for the
        # first score exps.
        emit_kq("k", 0, act=True)
        emit_kq("q", 0, act=True)
        for tk in range(5):
            emit_v(tk)

        # Projections/Wo interleaved between attention groups. K cc must
        # complete before heads 2cc/2cc+1 of the NEXT qc region use it; k8
        # (keys 1024:1152) before qc1; kr (1152:1536) + Q tb2 before qc2.
        ILV = {
            (0, 0): [("kq2", "k", 1)],
            (0, 1): [("kq2", "q", 1)],
            (0, 2): [("kq2", "k", 2)],
            (0, 3): [("kq2", "q", 2)],
            (0, 4): [("kq2", "k", 3)],
            (0, 5): [("kq2", "q", 3), ("v", 5)],
            (0, 6): [("v", 6), ("v", 7), ("k2", 0)],
            (0, 7): [("v", 8), ("k2", 1), ("k2", 2), ("k2", 3)],
            (1, 0): [("v", 9)],
            (1, 1): [("v", 10)],
            (1, 2): [("v", 11)],
            (1, 3): [("q2", 0), ("p3", 0, (0, 2))],
            (1, 4): [("q2", 1), ("p3", 0, (2, 4))],
            (1, 5): [("q2", 2), ("p3", 0, (4, 6))],
            (1, 6): [("q2", 3), ("p3", 0, (6, 8)), ("out", 0, (0, 8))],
            (1, 7): [("p3", 1, (0, 2))],
            (2, 0): [("p3", 1, (2, 4))],
            (2, 1): [("p3", 1, (4, 6))],
            (2, 2): [("p3", 1, (6, 8)), ("out", 1, (0, 8))],
        }

        for qc in range(NQC):
            # qc2 runs heads 0,1 (Wo plane cc0) last so the tail Wo pass can
            # accumulate cc1-3 while their normalize chains drain.
            order = (2, 3, 4, 5, 6, 7, 0, 1) if qc == NQC - 1 else range(HL)
            for pos_i, h in enumerate(order):
                attn(h, qc)
                for unit in ILV.get((qc, pos_i), []):
                    kind = unit[0]
                    if kind == "kq2":
                        emit_kq(unit[1], unit[2])
                    elif kind == "k2":
                        emit_k2(unit[1])
                    elif kind == "q2":
                        emit_q_range(unit[1], 1024, TQ)
                    elif kind == "p3":
                        phase3(unit[1], range(*unit[2]))
                    elif kind == "out":
                        dma_out(unit[1], *unit[2])
                    else:
                        emit_v(unit[1])
            if qc == NQC - 1:
                phase3_tail(qc)

    nc.finalize()
    return nc


_NC = None


def _get_nc():
    global _NC
    if _NC is None:
        _NC = _build()
    return _NC


def _make_in_maps(hidden_states, Wq, bq, Wk, Wv, Wo):
    hs = np.ascontiguousarray(hidden_states, np.float32)
    Wq = np.asarray(Wq, np.float32)
    Wk = np.asarray(Wk, np.float32)
    Wv = np.asarray(Wv, np.float32)
    Wo = np.asarray(Wo, np.float32)
    bq = np.asarray(bq, np.float32)

    bf = ml_dtypes.bfloat16
    mask_arr = np.ascontiguousarray(np.stack(_MASKS)).astype(bf)
    wq_s = Wq * np.float32(SCALE)
    vones_bf = _VONES.astype(bf)

    in_maps = []
    for core in range(NCORES):
        b, hh = core // 2, core % 2
        sl = slice(hh * CH, (hh + 1) * CH)
        hsT_pad = np.zeros((D, TP), np.float32)
        hsT_pad[:, :T] = hs[b].T
        in_maps.append({
            "hsT": hsT_pad.astype(bf),
            "wq": np.ascontiguousarray(wq_s[:, sl]).astype(bf),
            "wk": np.ascontiguousarray(Wk[:, sl]).astype(bf),
            "wv": np.ascontiguousarray(Wv[:, sl]).astype(bf),
            "bqs": np.ascontiguousarray(bq[sl] * np.float32(SCALE)),
            "wo": np.ascontiguousarray(Wo[sl, :]).astype(bf),
            "maskT": mask_arr,
            "vones": vones_bf,
        })
    return in_maps


def _assemble(results, bv, Wo, bo):
    c0 = (np.asarray(bv, np.float32) @ np.asarray(Wo, np.float32)
          + np.asarray(bo, np.float32))
    out = np.empty((B, T, D), np.float32)
    for b in range(B):
        out[b] = (results[2 * b]["outT"][:, :T].astype(np.float32).T
                  + results[2 * b + 1]["outT"][:, :T].astype(np.float32).T
                  + c0)
    return out


def kernel(hidden_states, Wq, bq, Wk, Wv, bv, Wo, bo):
    in_maps = _make_in_maps(hidden_states, Wq, bq, Wk, Wv, Wo)
    res = bass_utils.run_bass_kernel_spmd(
        _get_nc(), in_maps, core_ids=list(range(NCORES))
    )
    return _assemble(res.results, bv, Wo, bo)
